# revision 47
# baseline (speedup 1.0000x reference)
"""minGRU cell kernel for 8 Trainium2 NeuronCores.

Math (per batch b, all in linear domain — the recurrence is a convex
combination of positive values, so no log-space is needed):
    gh[s, :] = x[s, :] @ W.T + b          # (S, 2H)
    gate, hidden = gh[:, :H], gh[:, H:]
    z = sigmoid(gate);  a = 1 - z = sigmoid(-gate)
    g(hidden) = relu(hidden) + min(sigmoid(hidden), 0.5)
    h_t = a_t * h_{t-1} + z_t * g_t       # scan over s

Distribution: pure data parallel over B (8 batches -> 8 cores).

Device layout: channels on SBUF partitions, time on the free dim:
    out[o, s] = sum_i WT[i, o] * xT[i, s]
so the matmul result lands directly in the layout the DVE
tensor_tensor_scan instruction needs.  x and W are fed to the PE in
float16 (fp32 accumulate in PSUM): same 1 column/cycle rate as
float32r, but half the SBUF stream bytes, which lets the matmul
stream run at the full 2.4 GHz cadence (fp32r rows + fp32 LDWEIGHTS
oversubscribe SBUF read bandwidth and degrade the cadence ~15%), and
half the HBM traffic for the DMA-bound startup.  Matmul rounding in
fp16 lands at 2.2e-3 max rel err vs the 2e-2 gate (bf16 fails at
2.3e-2).

Perf structure (vs the naive version):
  * W is packed host-side per output-tile (o-major), so the first
    j-chain only needs 1 MB of W + 2.1 MB of x before the PE can
    start — the old k-major layout needed all 8.4 MB of W first
    (26 us of PE idle at startup).
  * DMA descriptor issue is spread across three queues (W on the
    Vector queue, x on Sync, consts + output stores on GpSimd) so
    descriptor serialization never gates the startup transfers.
  * Within each (chunk, j) iteration the hidden chain runs BEFORE the
    gate chain: the post-matmul serial tail (sigmoid/relu/min-add on
    the hidden projection) overlaps the gate matmuls.  For the very
    last chain, the hidden half is hoisted to the top of the final
    chunk (ACT/DVE queues are strict FIFO, so its sg/relu would
    otherwise serialize ahead of the tail), the gate matmuls run as
    two 256-column blocks with SEPARATE psum tiles (psum dependencies
    are whole-tile granular, so a shared tile would make the first
    block's z wait for all 16 matmuls), and a = 1-z runs on the DVE --
    the post-last-matmul serial tail is just z -> a/b -> scan -> store.
"""

from contextlib import ExitStack

import numpy as np

import concourse.bass as bass
import concourse.bacc as bacc
import concourse.mybir as mybir
import concourse.tile as tile
from concourse.bass_utils import run_bass_kernel_spmd

B, S, DIN, DH = 8, 4096, 1024, 1024
CH = 512                 # time-chunk (free dim of each matmul / scan)
NCHUNK = S // CH         # 8
KT = DIN // 128          # 8 contraction tiles
JT = DH // 128           # 8 channel tiles (per gate/hidden half)

F32 = mybir.dt.float32
F16 = mybir.dt.float16
AF = mybir.ActivationFunctionType
OP = mybir.AluOpType

_prog_cache = {}


def _build_program() -> bass.Bass:
    nc = bacc.Bacc("TRN2", target_bir_lowering=False, debug=False,
                   num_devices=B)
    xt = nc.dram_tensor("xt", (KT, 128, S), F16, kind="ExternalInput")
    # per-o packed weights: wp[o*128+p, k*128+c] = W[o*128+c, k*128+p]
    wp = nc.dram_tensor("wp", (2 * DH, DIN), F16, kind="ExternalInput")
    bias = nc.dram_tensor("bias", (128, 2 * JT), F32, kind="ExternalInput")
    nbias = nc.dram_tensor("nbias", (128, 2 * JT), F32, kind="ExternalInput")
    h0 = nc.dram_tensor("h0", (128, JT), F32, kind="ExternalInput")
    out = nc.dram_tensor("out", (DH, S), F32, kind="ExternalOutput")

    with ExitStack() as ctx:
        tc = ctx.enter_context(tile.TileContext(nc))
        cpool = ctx.enter_context(tc.tile_pool(name="const", bufs=1))
        wpool = ctx.enter_context(tc.tile_pool(name="w", bufs=1))
        xpool = ctx.enter_context(tc.tile_pool(name="x", bufs=2))
        spool = ctx.enter_context(tc.tile_pool(name="tmp", bufs=2))
        abpool = ctx.enter_context(tc.tile_pool(name="ab", bufs=3))
        hpool = ctx.enter_context(tc.tile_pool(name="h", bufs=2))
        # 2 tiles per chain, bufs=4 => exactly 2 chains in flight.  More
        # (bufs=6/8) lets the PE run further ahead of ACT, and the
        # then-permanent concurrent ACT psum reads slow the matmul
        # accumulation cadence (measured both in the fp32r and fp16
        # versions).
        ppool = ctx.enter_context(tc.tile_pool(name="psum", bufs=4, space="PSUM"))

        # Consts go over the (otherwise idle) GpSimd queue, then are
        # re-materialized on the engines that consume them (ACT for
        # bias/nbias, DVE for h0) so hot-loop instructions never carry
        # a DMA sync-wait.
        bias_d = cpool.tile([128, 2 * JT], F32, tag="bias_d")
        nc.gpsimd.dma_start(bias_d[:], bias[:, :])
        nbias_d = cpool.tile([128, 2 * JT], F32, tag="nbias_d")
        nc.gpsimd.dma_start(nbias_d[:], nbias[:, :])
        h0_d = cpool.tile([128, JT], F32, tag="h0_d")
        nc.gpsimd.dma_start(h0_d[:], h0[:, :])

        # Startup is DMA-bandwidth-bound: ~12.6 MB (W + x chunks 0/1)
        # must land in the first ~40 us.  Startup-critical input
        # transfers are sequenced in exact need order so later
        # transfers never steal bandwidth from earlier ones:
        #   w(j=0), x chunk 0, w(j=1..5), x chunk 1, w(j=6,7)
        # The first batch is split across the Sync AND Scalar queues
        # (two descriptors in flight ramps the DMA engines up faster);
        # everything after runs on Sync alone so the Scalar queue is
        # free for the ACT hot loop by the time psum drains start.
        def w_load(o, queue=nc.sync):
            w_t = wpool.tile([128, DIN], F16, tag=f"w{o}")
            queue.dma_start(w_t[:], wp[o * 128:(o + 1) * 128, :])
            wts[o] = w_t

        def x_load(c, queues=None):
            # per-k descriptors: chunk-0 chains consume tiles
            # k-progressively as they land.
            s0 = c * CH
            xts = []
            for k in range(KT):
                x_t = xpool.tile([128, CH], F16, tag=f"x{k}")
                q = queues[k % len(queues)] if queues else nc.sync
                q.dma_start(x_t[:], xt[k, :, s0:s0 + CH])
                xts.append(x_t)
            xts_by_chunk[s0] = [
                (lambda t: (lambda lo, hi: t[:, lo:hi]))(x_t) for x_t in xts]

        def x_load_big(s0, w):
            # steady-state chunks: ONE descriptor for the whole chunk
            # (8 strided k-blocks) => one DMA semaphore instead of 8 on
            # the Tensor queue.
            xbig = xpool.tile([128, KT * CH], F16, tag="xbig")
            nc.sync.dma_start(
                xbig[:, :KT * w].rearrange("p (k t) -> p k t", k=KT),
                xt[:, :, s0:s0 + w].rearrange("k p t -> p k t"))
            xts_by_chunk[s0] = [
                (lambda kk: (lambda lo, hi:
                             xbig[:, kk * w + lo:kk * w + hi]))(k)
                for k in range(KT)]

        wts = [None] * (2 * JT)
        xts_by_chunk = {}
        # Chunk-0's end is data-bound (w pair + ~1.3 MB) no matter how
        # early the first matmul fires, and the DMA queues ramp slowly in
        # the first few us, so keep the simple smooth split: w tiles whole
        # on Sync, x chunk 0 alternating between Sync and Scalar, gate
        # w(j=0) on Scalar.  (Tried and regressed: per-k W slices [256 B
        # runs collapse DMA efficiency], x0.k0-first-on-Scalar with w(0)
        # on GpSimd [Sync ramp starves the even x tiles].)
        w_load(JT + 0)
        w_load(0, nc.scalar)
        # Chunk-0 x tiles round-robin over THREE queues: GpSimd has only
        # ~12 KB of consts queued, so its tiles (k=2,5) land first and
        # the Sync/Scalar tiles are not serialized behind the w tiles as
        # deeply.
        x_load(0, queues=[nc.sync, nc.scalar, nc.gpsimd])
        for j in range(1, 6):
            w_load(JT + j), w_load(j)
        x_load(1)
        for j in range(6, 8):
            w_load(JT + j), w_load(j)

        # Const copies: first ACT op needs bias_t at ~17 us.
        bias_t = cpool.tile([128, 2 * JT], F32, tag="bias")
        nc.scalar.copy(bias_t[:], bias_d[:])
        nbias_t = cpool.tile([128, 2 * JT], F32, tag="nbias")
        nc.scalar.copy(nbias_t[:], nbias_d[:])
        h0_t = cpool.tile([128, JT], F32, tag="h0")
        nc.vector.tensor_copy(h0_t[:], h0_d[:])

        prev_h = [None] * JT
        g7pool = ctx.enter_context(tc.tile_pool(name="g7", bufs=1))

        def hidden_part(s0, w, j, sub, pool=None):
            """Hidden-half matmuls + post-ops (sg/relu/g) for the chain
            over time columns [s0, s0+w).  Returns a g-lookup closure.
            `pool` pins the g tiles in a dedicated pool so they survive
            the spool rotation (used to hoist the last chain's hidden
            work out of the end-of-kernel ACT queue)."""
            xl = xts_by_chunk[s0]
            ph = ppool.tile([128, CH], F32, tag="psum", name="ph")
            for k in range(KT):
                nc.tensor.matmul(
                    ph[:, :w],
                    lhsT=wts[JT + j][:, k * 128:(k + 1) * 128],
                    rhs=xl[k](0, w),
                    start=(k == 0),
                    stop=(k == KT - 1),
                )
            tiles = []
            for f0 in range(0, w, sub):
                fs = slice(f0, f0 + sub)
                sg_t = spool.tile([128, sub], F32, tag="sg")
                nc.scalar.activation(sg_t[:], ph[:, fs], AF.Sigmoid,
                                     bias=bias_t[:, JT + j:JT + j + 1],
                                     scale=1.0)
                r_t = spool.tile([128, sub], F32, tag="r")
                nc.scalar.activation(r_t[:], ph[:, fs], AF.Relu,
                                     bias=bias_t[:, JT + j:JT + j + 1],
                                     scale=1.0)
                # g = min(sigmoid(hidden), 0.5) + relu(hidden)
                if pool is None:
                    g_t = spool.tile([128, sub], F32, tag="g")
                else:
                    g_t = pool.tile([128, sub], F32, tag=f"g7_{f0}",
                                    name="g7_t")
                nc.vector.scalar_tensor_tensor(g_t[:], sg_t[:], 0.5,
                                               r_t[:], op0=OP.min,
                                               op1=OP.add)
                tiles.append((f0, sub, g_t))

            def g_at(f0, w):
                for t0, tw, t in tiles:
                    if t0 <= f0 and f0 + w <= t0 + tw:
                        return t[:, f0 - t0:f0 - t0 + w]
                raise KeyError((f0, w))
            return g_at

        def gate_part(s0, w, j, sub, g_at, gate_split=1, a_on_dve=False,
                      scan_q=None):
            """Gate-half matmuls + a/z/b/scan/store for the chain over
            time columns [s0, s0+w).  `gate_split` > 1 runs the gate
            matmuls as that many column sub-chains so the earlier blocks'
            post-ops overlap the later blocks' matmuls, and `a_on_dve`
            computes a = 1-z on the DVE instead of a second ACT sigmoid
            (both shorten the serial post-matmul tail; used for the very
            last chain only)."""
            xl = xts_by_chunk[s0]
            pg = ppool.tile([128, CH], F32, tag="psum", name="pg")
            gw = w // gate_split
            for lo in range(0, w, gw):
                for k in range(KT):
                    nc.tensor.matmul(
                        pg[:, lo:lo + gw],
                        lhsT=wts[j][:, k * 128:(k + 1) * 128],
                        rhs=xl[k](lo, lo + gw),
                        start=(k == 0),
                        stop=(k == KT - 1),
                    )
            h_t = hpool.tile([128, CH], F32, tag=f"h{j}")
            for f0 in range(0, w, sub):
                fs = slice(f0, f0 + sub)
                z_t = spool.tile([128, sub], F32, tag="z")
                nc.scalar.activation(z_t[:], pg[:, fs], AF.Sigmoid,
                                     bias=bias_t[:, j:j + 1], scale=1.0)
                a_t = abpool.tile([128, sub], F32, tag="a")
                if a_on_dve:
                    nc.vector.tensor_scalar(a_t[:], z_t[:], -1.0, 1.0,
                                            op0=OP.mult, op1=OP.add)
                else:
                    nc.scalar.activation(a_t[:], pg[:, fs], AF.Sigmoid,
                                         bias=nbias_t[:, j:j + 1], scale=-1.0)
                b_t = abpool.tile([128, sub], F32, tag="b")
                nc.vector.tensor_mul(b_t[:], z_t[:], g_at(f0, sub))
                # ---- scan: h = a*h_prev + b along time
                if f0 == 0:
                    if s0 == 0:
                        init = h0_t[:, j:j + 1]
                    else:
                        pt, pw = prev_h[j]
                        init = pt[:, pw - 1:pw]
                else:
                    init = h_t[:, f0 - 1:f0]
                (scan_q or nc.vector).tensor_tensor_scan(
                    h_t[:, fs], a_t[:], b_t[:], init,
                    op0=OP.mult, op1=OP.add)
            prev_h[j] = (h_t, w)
            # GpSimd's end-of-kernel DRAIN detects DMA completion
            # slowly (~6 us); keep the final chunks' stores on Sync
            # (idle by then) so the kernel end isn't gated on it.
            # The final chunk stores per-sub so the last transfer
            # is short.
            last = s0 + w == S
            out_q = nc.gpsimd if s0 + w <= S - 2 * CH else nc.sync
            if last:
                for f0 in range(0, w, sub):
                    out_q.dma_start(
                        out[j * 128:(j + 1) * 128, s0 + f0:s0 + f0 + sub],
                        h_t[:, f0:f0 + sub])
            else:
                out_q.dma_start(out[j * 128:(j + 1) * 128, s0:s0 + w],
                                h_t[:, :w])

        def gate_part_tail(s0, w, j, g_at):
            """Final chain's gate half: each 256-column block gets its OWN
            psum tile (with a shared tile the first block's z waits for
            ALL 16 matmuls -- psum dependencies are whole-tile granular),
            so block 1's z/a/b/scan overlap block 2's matmuls.  a = 1-z
            runs on the DVE; everything after the last matmul is one
            z -> a/b -> scan -> store chain."""
            xl = xts_by_chunk[s0]
            h_t = hpool.tile([128, CH], F32, tag=f"h{j}")
            # Asymmetric blocks: the final block is only 128 columns so
            # the serial chain after the very last matmul is as short as
            # possible; the 384-block's post-ops run under the 128-block's
            # matmuls.
            blocks = ((0, 3 * w // 4), (3 * w // 4, w // 4))
            for hi, (lo, bw) in enumerate(blocks):
                pg = ppool.tile([128, CH], F32, tag="psum", name=f"pgt{hi}")
                for k in range(KT):
                    nc.tensor.matmul(
                        pg[:, :bw],
                        lhsT=wts[j][:, k * 128:(k + 1) * 128],
                        rhs=xl[k](lo, lo + bw),
                        start=(k == 0),
                        stop=(k == KT - 1),
                    )
                z_t = spool.tile([128, bw], F32, tag="z")
                nc.scalar.activation(z_t[:], pg[:, :bw], AF.Sigmoid,
                                     bias=bias_t[:, j:j + 1], scale=1.0)
                a_t = abpool.tile([128, bw], F32, tag="a")
                nc.vector.tensor_scalar(a_t[:], z_t[:], -1.0, 1.0,
                                        op0=OP.mult, op1=OP.add)
                b_t = abpool.tile([128, bw], F32, tag="b")
                nc.vector.tensor_mul(b_t[:], z_t[:], g_at(lo, bw))
                if lo == 0:
                    pt, pw = prev_h[j]
                    init = pt[:, pw - 1:pw]
                else:
                    init = h_t[:, lo - 1:lo]
                nc.vector.tensor_tensor_scan(h_t[:, lo:lo + bw], a_t[:],
                                             b_t[:], init, op0=OP.mult,
                                             op1=OP.add)
                nc.sync.dma_start(
                    out[j * 128:(j + 1) * 128, s0 + lo:s0 + lo + bw],
                    h_t[:, lo:lo + bw])
            prev_h[j] = (h_t, w)

        def chain(s0, w, j, sub, gate_split=1, a_on_dve=False,
                  scan_q=None):
            gate_part(s0, w, j, sub, hidden_part(s0, w, j, sub),
                      gate_split, a_on_dve, scan_q)

        # Chunks 0/1 (512 wide): single-chunk chains, interleaved so
        # chains line up with the W/x arrival schedule.
        for c, j in ([(0, j) for j in range(6)] + [(1, j) for j in range(4)]
                     + [(0, 6), (0, 7)] + [(1, j) for j in range(4, 8)]):
            chain(c * CH, CH, j, CH)
        # Steady-state 512-wide chunks (the ISA rejects matmul outputs
        # > 512 elements -- s3d3_mm_num_elements -- so wider chunks are
        # not possible).  The last chain splits its gate matmuls and
        # post-ops so the serial a/z/mul/scan/store tail after the last
        # matmul is short.
        for s0, w in [(c * CH, CH) for c in range(2, NCHUNK)]:
            x_load_big(s0, w)
            if s0 + w == S:
                # Final chunk: hoist the last chain's hidden half to the
                # front (its sg/relu would otherwise sit in the ACT FIFO
                # ahead of the tail z ops), then run its gate half last
                # with per-block psum tiles.
                g7 = hidden_part(s0, w, JT - 1, CH, pool=g7pool)
                for j in range(JT - 1):
                    chain(s0, w, j, CH)
                gate_part_tail(s0, w, JT - 1, g7)
            else:
                for j in range(JT):
                    chain(s0, w, j, CH)

    nc.compile()
    return nc


def _run(inputs, trace=False, **spmd_kwargs):
    x = np.asarray(inputs["x"], dtype=np.float32)
    h = np.asarray(inputs["h"], dtype=np.float32)
    W = np.asarray(inputs["W"], dtype=np.float32)
    b = np.asarray(inputs["b"], dtype=np.float32)

    xt_all = np.ascontiguousarray(x.transpose(0, 2, 1)).astype(np.float16).reshape(
        B, KT, 128, S)                                             # (B,KT,128,S)
    # wp[o*128+p, k*128+c] = W[o*128+c, k*128+p]
    WP = np.ascontiguousarray(
        W.reshape(2 * JT, 128, KT, 128).transpose(0, 3, 2, 1)
        .reshape(2 * DH, DIN)).astype(np.float16)
    bias_t = np.ascontiguousarray(b.reshape(2 * JT, 128).T)        # (128, 2JT)
    nbias_t = np.ascontiguousarray(-bias_t)
    h0_all = np.ascontiguousarray(
        h[:, 0, :].reshape(B, JT, 128).transpose(0, 2, 1))         # (B, 128, JT)

    if "prog" not in _prog_cache:
        _prog_cache["prog"] = _build_program()
    nc = _prog_cache["prog"]

    in_maps = [
        {"xt": xt_all[c], "wp": WP, "bias": bias_t, "nbias": nbias_t,
         "h0": h0_all[c]}
        for c in range(B)
    ]
    res = run_bass_kernel_spmd(nc, in_maps, list(range(B)), trace=trace,
                               **spmd_kwargs)
    out = np.stack([res.results[c]["out"].T for c in range(B)], axis=0)
    return np.ascontiguousarray(out), res


def kernel(**inputs) -> np.ndarray:
    return _run(inputs)[0]



# revision 48
# speedup vs baseline: 1.0044x; 1.0044x over previous
"""minGRU cell kernel for 8 Trainium2 NeuronCores.

Math (per batch b, all in linear domain — the recurrence is a convex
combination of positive values, so no log-space is needed):
    gh[s, :] = x[s, :] @ W.T + b          # (S, 2H)
    gate, hidden = gh[:, :H], gh[:, H:]
    z = sigmoid(gate);  a = 1 - z = sigmoid(-gate)
    g(hidden) = relu(hidden) + min(sigmoid(hidden), 0.5)
    h_t = a_t * h_{t-1} + z_t * g_t       # scan over s

Distribution: pure data parallel over B (8 batches -> 8 cores).

Device layout: channels on SBUF partitions, time on the free dim:
    out[o, s] = sum_i WT[i, o] * xT[i, s]
so the matmul result lands directly in the layout the DVE
tensor_tensor_scan instruction needs.  x and W are fed to the PE in
float16 (fp32 accumulate in PSUM): same 1 column/cycle rate as
float32r, but half the SBUF stream bytes, which lets the matmul
stream run at the full 2.4 GHz cadence (fp32r rows + fp32 LDWEIGHTS
oversubscribe SBUF read bandwidth and degrade the cadence ~15%), and
half the HBM traffic for the DMA-bound startup.  Matmul rounding in
fp16 lands at 2.2e-3 max rel err vs the 2e-2 gate (bf16 fails at
2.3e-2).

Perf structure (vs the naive version):
  * W is packed host-side per output-tile (o-major), so the first
    j-chain only needs 1 MB of W + 2.1 MB of x before the PE can
    start — the old k-major layout needed all 8.4 MB of W first
    (26 us of PE idle at startup).
  * DMA descriptor issue is spread across three queues (W on the
    Vector queue, x on Sync, consts + output stores on GpSimd) so
    descriptor serialization never gates the startup transfers.
  * Within each (chunk, j) iteration the hidden chain runs BEFORE the
    gate chain: the post-matmul serial tail (sigmoid/relu/min-add on
    the hidden projection) overlaps the gate matmuls.  For the very
    last chain, the hidden half is hoisted to the top of the final
    chunk (ACT/DVE queues are strict FIFO, so its sg/relu would
    otherwise serialize ahead of the tail), the gate matmuls run as
    two 256-column blocks with SEPARATE psum tiles (psum dependencies
    are whole-tile granular, so a shared tile would make the first
    block's z wait for all 16 matmuls), and a = 1-z runs on the DVE --
    the post-last-matmul serial tail is just z -> a/b -> scan -> store.
"""

from contextlib import ExitStack

import numpy as np

import concourse.bass as bass
import concourse.bacc as bacc
import concourse.mybir as mybir
import concourse.tile as tile
from concourse.bass_utils import run_bass_kernel_spmd

B, S, DIN, DH = 8, 4096, 1024, 1024
CH = 512                 # time-chunk (free dim of each matmul / scan)
NCHUNK = S // CH         # 8
KT = DIN // 128          # 8 contraction tiles
JT = DH // 128           # 8 channel tiles (per gate/hidden half)

F32 = mybir.dt.float32
F16 = mybir.dt.float16
AF = mybir.ActivationFunctionType
OP = mybir.AluOpType

_prog_cache = {}


def _build_program() -> bass.Bass:
    nc = bacc.Bacc("TRN2", target_bir_lowering=False, debug=False,
                   num_devices=B)
    xt = nc.dram_tensor("xt", (KT, 128, S), F16, kind="ExternalInput")
    # per-o packed weights: wp[o*128+p, k*128+c] = W[o*128+c, k*128+p]
    wp = nc.dram_tensor("wp", (2 * DH, DIN), F16, kind="ExternalInput")
    bias = nc.dram_tensor("bias", (128, 2 * JT), F32, kind="ExternalInput")
    nbias = nc.dram_tensor("nbias", (128, 2 * JT), F32, kind="ExternalInput")
    h0 = nc.dram_tensor("h0", (128, JT), F32, kind="ExternalInput")
    out = nc.dram_tensor("out", (DH, S), F32, kind="ExternalOutput")

    with ExitStack() as ctx:
        tc = ctx.enter_context(tile.TileContext(nc))
        cpool = ctx.enter_context(tc.tile_pool(name="const", bufs=1))
        wpool = ctx.enter_context(tc.tile_pool(name="w", bufs=1))
        xpool = ctx.enter_context(tc.tile_pool(name="x", bufs=2))
        spool = ctx.enter_context(tc.tile_pool(name="tmp", bufs=2))
        abpool = ctx.enter_context(tc.tile_pool(name="ab", bufs=3))
        hpool = ctx.enter_context(tc.tile_pool(name="h", bufs=2))
        # 2 tiles per chain, bufs=4 => exactly 2 chains in flight.  More
        # (bufs=6/8) lets the PE run further ahead of ACT, and the
        # then-permanent concurrent ACT psum reads slow the matmul
        # accumulation cadence (measured both in the fp32r and fp16
        # versions).
        ppool = ctx.enter_context(tc.tile_pool(name="psum", bufs=4, space="PSUM"))

        # Consts go over the (otherwise idle) GpSimd queue, then are
        # re-materialized on the engines that consume them (ACT for
        # bias/nbias, DVE for h0) so hot-loop instructions never carry
        # a DMA sync-wait.
        bias_d = cpool.tile([128, 2 * JT], F32, tag="bias_d")
        nc.gpsimd.dma_start(bias_d[:], bias[:, :])
        nbias_d = cpool.tile([128, 2 * JT], F32, tag="nbias_d")
        nc.gpsimd.dma_start(nbias_d[:], nbias[:, :])
        h0_d = cpool.tile([128, JT], F32, tag="h0_d")
        nc.gpsimd.dma_start(h0_d[:], h0[:, :])

        # Startup is DMA-bandwidth-bound: ~12.6 MB (W + x chunks 0/1)
        # must land in the first ~40 us.  Startup-critical input
        # transfers are sequenced in exact need order so later
        # transfers never steal bandwidth from earlier ones:
        #   w(j=0), x chunk 0, w(j=1..5), x chunk 1, w(j=6,7)
        # The first batch is split across the Sync AND Scalar queues
        # (two descriptors in flight ramps the DMA engines up faster);
        # everything after runs on Sync alone so the Scalar queue is
        # free for the ACT hot loop by the time psum drains start.
        def w_load(o, queue=nc.sync):
            w_t = wpool.tile([128, DIN], F16, tag=f"w{o}")
            queue.dma_start(w_t[:], wp[o * 128:(o + 1) * 128, :])
            wts[o] = w_t

        def x_load(c, queues=None):
            # per-k descriptors: chunk-0 chains consume tiles
            # k-progressively as they land.
            s0 = c * CH
            xts = []
            for k in range(KT):
                x_t = xpool.tile([128, CH], F16, tag=f"x{k}")
                q = queues[k % len(queues)] if queues else nc.sync
                q.dma_start(x_t[:], xt[k, :, s0:s0 + CH])
                xts.append(x_t)
            xts_by_chunk[s0] = [
                (lambda t: (lambda lo, hi: t[:, lo:hi]))(x_t) for x_t in xts]

        def x_load_big(s0, w):
            # steady-state chunks: ONE descriptor for the whole chunk
            # (8 strided k-blocks) => one DMA semaphore instead of 8 on
            # the Tensor queue.
            xbig = xpool.tile([128, KT * CH], F16, tag="xbig")
            nc.sync.dma_start(
                xbig[:, :KT * w].rearrange("p (k t) -> p k t", k=KT),
                xt[:, :, s0:s0 + w].rearrange("k p t -> p k t"))
            xts_by_chunk[s0] = [
                (lambda kk: (lambda lo, hi:
                             xbig[:, kk * w + lo:kk * w + hi]))(k)
                for k in range(KT)]

        wts = [None] * (2 * JT)
        xts_by_chunk = {}
        # Chunk-0's end is data-bound (w pair + ~1.3 MB) no matter how
        # early the first matmul fires, and the DMA queues ramp slowly in
        # the first few us, so keep the simple smooth split: w tiles whole
        # on Sync, x chunk 0 alternating between Sync and Scalar, gate
        # w(j=0) on Scalar.  (Tried and regressed: per-k W slices [256 B
        # runs collapse DMA efficiency], x0.k0-first-on-Scalar with w(0)
        # on GpSimd [Sync ramp starves the even x tiles].)
        w_load(JT + 0)
        w_load(0, nc.scalar)
        # (A third queue for chunk-0 x was tried and regresses: the 8 DMA
        # engines are shared across queues, so it steals from Sync/Scalar.)
        x_load(0, queues=[nc.sync, nc.scalar])
        for j in range(1, 6):
            w_load(JT + j), w_load(j)
        x_load(1)
        for j in range(6, 8):
            w_load(JT + j), w_load(j)

        # Const copies: first ACT op needs bias_t at ~17 us.
        bias_t = cpool.tile([128, 2 * JT], F32, tag="bias")
        nc.scalar.copy(bias_t[:], bias_d[:])
        nbias_t = cpool.tile([128, 2 * JT], F32, tag="nbias")
        nc.scalar.copy(nbias_t[:], nbias_d[:])
        h0_t = cpool.tile([128, JT], F32, tag="h0")
        nc.vector.tensor_copy(h0_t[:], h0_d[:])

        prev_h = [None] * JT
        g7pool = ctx.enter_context(tc.tile_pool(name="g7", bufs=1))

        def hidden_part(s0, w, j, sub, pool=None):
            """Hidden-half matmuls + post-ops (sg/relu/g) for the chain
            over time columns [s0, s0+w).  Returns a g-lookup closure.
            `pool` pins the g tiles in a dedicated pool so they survive
            the spool rotation (used to hoist the last chain's hidden
            work out of the end-of-kernel ACT queue)."""
            xl = xts_by_chunk[s0]
            ph = ppool.tile([128, CH], F32, tag="psum", name="ph")
            for k in range(KT):
                nc.tensor.matmul(
                    ph[:, :w],
                    lhsT=wts[JT + j][:, k * 128:(k + 1) * 128],
                    rhs=xl[k](0, w),
                    start=(k == 0),
                    stop=(k == KT - 1),
                )
            tiles = []
            for f0 in range(0, w, sub):
                fs = slice(f0, f0 + sub)
                sg_t = spool.tile([128, sub], F32, tag="sg")
                nc.scalar.activation(sg_t[:], ph[:, fs], AF.Sigmoid,
                                     bias=bias_t[:, JT + j:JT + j + 1],
                                     scale=1.0)
                r_t = spool.tile([128, sub], F32, tag="r")
                nc.scalar.activation(r_t[:], ph[:, fs], AF.Relu,
                                     bias=bias_t[:, JT + j:JT + j + 1],
                                     scale=1.0)
                # g = min(sigmoid(hidden), 0.5) + relu(hidden)
                if pool is None:
                    g_t = spool.tile([128, sub], F32, tag="g")
                else:
                    g_t = pool.tile([128, sub], F32, tag=f"g7_{f0}",
                                    name="g7_t")
                nc.vector.scalar_tensor_tensor(g_t[:], sg_t[:], 0.5,
                                               r_t[:], op0=OP.min,
                                               op1=OP.add)
                tiles.append((f0, sub, g_t))

            def g_at(f0, w):
                for t0, tw, t in tiles:
                    if t0 <= f0 and f0 + w <= t0 + tw:
                        return t[:, f0 - t0:f0 - t0 + w]
                raise KeyError((f0, w))
            return g_at

        def gate_part(s0, w, j, sub, g_at, gate_split=1, a_on_dve=False,
                      scan_q=None):
            """Gate-half matmuls + a/z/b/scan/store for the chain over
            time columns [s0, s0+w).  `gate_split` > 1 runs the gate
            matmuls as that many column sub-chains so the earlier blocks'
            post-ops overlap the later blocks' matmuls, and `a_on_dve`
            computes a = 1-z on the DVE instead of a second ACT sigmoid
            (both shorten the serial post-matmul tail; used for the very
            last chain only)."""
            xl = xts_by_chunk[s0]
            pg = ppool.tile([128, CH], F32, tag="psum", name="pg")
            gw = w // gate_split
            for lo in range(0, w, gw):
                for k in range(KT):
                    nc.tensor.matmul(
                        pg[:, lo:lo + gw],
                        lhsT=wts[j][:, k * 128:(k + 1) * 128],
                        rhs=xl[k](lo, lo + gw),
                        start=(k == 0),
                        stop=(k == KT - 1),
                    )
            h_t = hpool.tile([128, CH], F32, tag=f"h{j}")
            for f0 in range(0, w, sub):
                fs = slice(f0, f0 + sub)
                z_t = spool.tile([128, sub], F32, tag="z")
                nc.scalar.activation(z_t[:], pg[:, fs], AF.Sigmoid,
                                     bias=bias_t[:, j:j + 1], scale=1.0)
                a_t = abpool.tile([128, sub], F32, tag="a")
                if a_on_dve:
                    nc.vector.tensor_scalar(a_t[:], z_t[:], -1.0, 1.0,
                                            op0=OP.mult, op1=OP.add)
                else:
                    nc.scalar.activation(a_t[:], pg[:, fs], AF.Sigmoid,
                                         bias=nbias_t[:, j:j + 1], scale=-1.0)
                b_t = abpool.tile([128, sub], F32, tag="b")
                nc.vector.tensor_mul(b_t[:], z_t[:], g_at(f0, sub))
                # ---- scan: h = a*h_prev + b along time
                if f0 == 0:
                    if s0 == 0:
                        init = h0_t[:, j:j + 1]
                    else:
                        pt, pw = prev_h[j]
                        init = pt[:, pw - 1:pw]
                else:
                    init = h_t[:, f0 - 1:f0]
                (scan_q or nc.vector).tensor_tensor_scan(
                    h_t[:, fs], a_t[:], b_t[:], init,
                    op0=OP.mult, op1=OP.add)
            prev_h[j] = (h_t, w)
            # GpSimd's end-of-kernel DRAIN detects DMA completion
            # slowly (~6 us); keep the final chunks' stores on Sync
            # (idle by then) so the kernel end isn't gated on it.
            # The final chunk stores per-sub so the last transfer
            # is short.
            last = s0 + w == S
            out_q = nc.gpsimd if s0 + w <= S - 2 * CH else nc.sync
            if last:
                for f0 in range(0, w, sub):
                    out_q.dma_start(
                        out[j * 128:(j + 1) * 128, s0 + f0:s0 + f0 + sub],
                        h_t[:, f0:f0 + sub])
            else:
                out_q.dma_start(out[j * 128:(j + 1) * 128, s0:s0 + w],
                                h_t[:, :w])

        def gate_part_tail(s0, w, j, g_at):
            """Final chain's gate half: each 256-column block gets its OWN
            psum tile (with a shared tile the first block's z waits for
            ALL 16 matmuls -- psum dependencies are whole-tile granular),
            so block 1's z/a/b/scan overlap block 2's matmuls.  a = 1-z
            runs on the DVE; everything after the last matmul is one
            z -> a/b -> scan -> store chain."""
            xl = xts_by_chunk[s0]
            h_t = hpool.tile([128, CH], F32, tag=f"h{j}")
            # Asymmetric blocks: the final block is only 128 columns so
            # the serial chain after the very last matmul is as short as
            # possible; the 384-block's post-ops run under the 128-block's
            # matmuls.
            blocks = ((0, 3 * w // 4), (3 * w // 4, w // 4))
            for hi, (lo, bw) in enumerate(blocks):
                pg = ppool.tile([128, CH], F32, tag="psum", name=f"pgt{hi}")
                for k in range(KT):
                    nc.tensor.matmul(
                        pg[:, :bw],
                        lhsT=wts[j][:, k * 128:(k + 1) * 128],
                        rhs=xl[k](lo, lo + bw),
                        start=(k == 0),
                        stop=(k == KT - 1),
                    )
                z_t = spool.tile([128, bw], F32, tag="z")
                nc.scalar.activation(z_t[:], pg[:, :bw], AF.Sigmoid,
                                     bias=bias_t[:, j:j + 1], scale=1.0)
                a_t = abpool.tile([128, bw], F32, tag="a")
                nc.vector.tensor_scalar(a_t[:], z_t[:], -1.0, 1.0,
                                        op0=OP.mult, op1=OP.add)
                b_t = abpool.tile([128, bw], F32, tag="b")
                nc.vector.tensor_mul(b_t[:], z_t[:], g_at(lo, bw))
                if lo == 0:
                    pt, pw = prev_h[j]
                    init = pt[:, pw - 1:pw]
                else:
                    init = h_t[:, lo - 1:lo]
                nc.vector.tensor_tensor_scan(h_t[:, lo:lo + bw], a_t[:],
                                             b_t[:], init, op0=OP.mult,
                                             op1=OP.add)
                nc.sync.dma_start(
                    out[j * 128:(j + 1) * 128, s0 + lo:s0 + lo + bw],
                    h_t[:, lo:lo + bw])
            prev_h[j] = (h_t, w)

        def chain(s0, w, j, sub, gate_split=1, a_on_dve=False,
                  scan_q=None):
            gate_part(s0, w, j, sub, hidden_part(s0, w, j, sub),
                      gate_split, a_on_dve, scan_q)

        # Chunks 0/1 (512 wide): single-chunk chains, interleaved so
        # chains line up with the W/x arrival schedule.
        for c, j in ([(0, j) for j in range(6)] + [(1, j) for j in range(4)]
                     + [(0, 6), (0, 7)] + [(1, j) for j in range(4, 8)]):
            chain(c * CH, CH, j, CH)
        # Steady-state 512-wide chunks (the ISA rejects matmul outputs
        # > 512 elements -- s3d3_mm_num_elements -- so wider chunks are
        # not possible).  The last chain splits its gate matmuls and
        # post-ops so the serial a/z/mul/scan/store tail after the last
        # matmul is short.
        for s0, w in [(c * CH, CH) for c in range(2, NCHUNK)]:
            x_load_big(s0, w)
            if s0 + w == S:
                # Final chunk: hoist the last chain's hidden half to the
                # front (its sg/relu would otherwise sit in the ACT FIFO
                # ahead of the tail z ops), then run its gate half last
                # with per-block psum tiles.
                g7 = hidden_part(s0, w, JT - 1, CH, pool=g7pool)
                for j in range(JT - 1):
                    chain(s0, w, j, CH)
                gate_part_tail(s0, w, JT - 1, g7)
            else:
                for j in range(JT):
                    chain(s0, w, j, CH)

    nc.compile()
    return nc


def _run(inputs, trace=False, **spmd_kwargs):
    x = np.asarray(inputs["x"], dtype=np.float32)
    h = np.asarray(inputs["h"], dtype=np.float32)
    W = np.asarray(inputs["W"], dtype=np.float32)
    b = np.asarray(inputs["b"], dtype=np.float32)

    xt_all = np.ascontiguousarray(x.transpose(0, 2, 1)).astype(np.float16).reshape(
        B, KT, 128, S)                                             # (B,KT,128,S)
    # wp[o*128+p, k*128+c] = W[o*128+c, k*128+p]
    WP = np.ascontiguousarray(
        W.reshape(2 * JT, 128, KT, 128).transpose(0, 3, 2, 1)
        .reshape(2 * DH, DIN)).astype(np.float16)
    bias_t = np.ascontiguousarray(b.reshape(2 * JT, 128).T)        # (128, 2JT)
    nbias_t = np.ascontiguousarray(-bias_t)
    h0_all = np.ascontiguousarray(
        h[:, 0, :].reshape(B, JT, 128).transpose(0, 2, 1))         # (B, 128, JT)

    if "prog" not in _prog_cache:
        _prog_cache["prog"] = _build_program()
    nc = _prog_cache["prog"]

    in_maps = [
        {"xt": xt_all[c], "wp": WP, "bias": bias_t, "nbias": nbias_t,
         "h0": h0_all[c]}
        for c in range(B)
    ]
    res = run_bass_kernel_spmd(nc, in_maps, list(range(B)), trace=trace,
                               **spmd_kwargs)
    out = np.stack([res.results[c]["out"].T for c in range(B)], axis=0)
    return np.ascontiguousarray(out), res


def kernel(**inputs) -> np.ndarray:
    return _run(inputs)[0]



# revision 49
# speedup vs baseline: 1.0060x; 1.0016x over previous
"""minGRU cell kernel for 8 Trainium2 NeuronCores.

Math (per batch b, all in linear domain — the recurrence is a convex
combination of positive values, so no log-space is needed):
    gh[s, :] = x[s, :] @ W.T + b          # (S, 2H)
    gate, hidden = gh[:, :H], gh[:, H:]
    z = sigmoid(gate);  a = 1 - z = sigmoid(-gate)
    g(hidden) = relu(hidden) + min(sigmoid(hidden), 0.5)
    h_t = a_t * h_{t-1} + z_t * g_t       # scan over s

Distribution: pure data parallel over B (8 batches -> 8 cores).

Device layout: channels on SBUF partitions, time on the free dim:
    out[o, s] = sum_i WT[i, o] * xT[i, s]
so the matmul result lands directly in the layout the DVE
tensor_tensor_scan instruction needs.  x and W are fed to the PE in
float16 (fp32 accumulate in PSUM): same 1 column/cycle rate as
float32r, but half the SBUF stream bytes, which lets the matmul
stream run at the full 2.4 GHz cadence (fp32r rows + fp32 LDWEIGHTS
oversubscribe SBUF read bandwidth and degrade the cadence ~15%), and
half the HBM traffic for the DMA-bound startup.  Matmul rounding in
fp16 lands at 2.2e-3 max rel err vs the 2e-2 gate (bf16 fails at
2.3e-2).

Perf structure (vs the naive version):
  * W is packed host-side per output-tile (o-major), so the first
    j-chain only needs 1 MB of W + 2.1 MB of x before the PE can
    start — the old k-major layout needed all 8.4 MB of W first
    (26 us of PE idle at startup).
  * DMA descriptor issue is spread across three queues (W on the
    Vector queue, x on Sync, consts + output stores on GpSimd) so
    descriptor serialization never gates the startup transfers.
  * Within each (chunk, j) iteration the hidden chain runs BEFORE the
    gate chain: the post-matmul serial tail (sigmoid/relu/min-add on
    the hidden projection) overlaps the gate matmuls.  For the very
    last chain, the hidden half is hoisted to the top of the final
    chunk (ACT/DVE queues are strict FIFO, so its sg/relu would
    otherwise serialize ahead of the tail), the gate matmuls run as
    384+128-column blocks with SEPARATE psum tiles (psum dependencies
    are whole-tile granular, so a shared tile would make the first
    block's z wait for all 16 matmuls), and a = 1-z runs on the DVE --
    the post-last-matmul serial tail is a 128-column
    z -> a/b -> scan -> store chain.
"""

from contextlib import ExitStack

import numpy as np

import concourse.bass as bass
import concourse.bacc as bacc
import concourse.mybir as mybir
import concourse.tile as tile
from concourse.bass_utils import run_bass_kernel_spmd

B, S, DIN, DH = 8, 4096, 1024, 1024
CH = 512                 # time-chunk (free dim of each matmul / scan)
NCHUNK = S // CH         # 8
KT = DIN // 128          # 8 contraction tiles
JT = DH // 128           # 8 channel tiles (per gate/hidden half)

F32 = mybir.dt.float32
F16 = mybir.dt.float16
AF = mybir.ActivationFunctionType
OP = mybir.AluOpType

_prog_cache = {}


def _build_program() -> bass.Bass:
    nc = bacc.Bacc("TRN2", target_bir_lowering=False, debug=False,
                   num_devices=B)
    xt = nc.dram_tensor("xt", (KT, 128, S), F16, kind="ExternalInput")
    # per-o packed weights: wp[o*128+p, k*128+c] = W[o*128+c, k*128+p]
    wp = nc.dram_tensor("wp", (2 * DH, DIN), F16, kind="ExternalInput")
    bias = nc.dram_tensor("bias", (128, 2 * JT), F32, kind="ExternalInput")
    nbias = nc.dram_tensor("nbias", (128, 2 * JT), F32, kind="ExternalInput")
    h0 = nc.dram_tensor("h0", (128, JT), F32, kind="ExternalInput")
    out = nc.dram_tensor("out", (DH, S), F32, kind="ExternalOutput")

    with ExitStack() as ctx:
        tc = ctx.enter_context(tile.TileContext(nc))
        cpool = ctx.enter_context(tc.tile_pool(name="const", bufs=1))
        wpool = ctx.enter_context(tc.tile_pool(name="w", bufs=1))
        xpool = ctx.enter_context(tc.tile_pool(name="x", bufs=2))
        spool = ctx.enter_context(tc.tile_pool(name="tmp", bufs=2))
        abpool = ctx.enter_context(tc.tile_pool(name="ab", bufs=3))
        hpool = ctx.enter_context(tc.tile_pool(name="h", bufs=2))
        # 2 tiles per chain, bufs=4 => exactly 2 chains in flight.  More
        # (bufs=6/8) lets the PE run further ahead of ACT, and the
        # then-permanent concurrent ACT psum reads slow the matmul
        # accumulation cadence (measured both in the fp32r and fp16
        # versions).
        ppool = ctx.enter_context(tc.tile_pool(name="psum", bufs=4, space="PSUM"))

        # Consts go over the (otherwise idle) GpSimd queue, then are
        # re-materialized on the engines that consume them (ACT for
        # bias/nbias, DVE for h0) so hot-loop instructions never carry
        # a DMA sync-wait.
        bias_d = cpool.tile([128, 2 * JT], F32, tag="bias_d")
        nc.gpsimd.dma_start(bias_d[:], bias[:, :])
        nbias_d = cpool.tile([128, 2 * JT], F32, tag="nbias_d")
        nc.gpsimd.dma_start(nbias_d[:], nbias[:, :])
        h0_d = cpool.tile([128, JT], F32, tag="h0_d")
        nc.gpsimd.dma_start(h0_d[:], h0[:, :])

        # Startup is DMA-bandwidth-bound: ~12.6 MB (W + x chunks 0/1)
        # must land in the first ~40 us.  Startup-critical input
        # transfers are sequenced in exact need order so later
        # transfers never steal bandwidth from earlier ones:
        #   w(j=0), x chunk 0, w(j=1..5), x chunk 1, w(j=6,7)
        # The first batch is split across the Sync AND Scalar queues
        # (two descriptors in flight ramps the DMA engines up faster);
        # everything after runs on Sync alone so the Scalar queue is
        # free for the ACT hot loop by the time psum drains start.
        def w_load(o, queue=nc.sync):
            w_t = wpool.tile([128, DIN], F16, tag=f"w{o}")
            queue.dma_start(w_t[:], wp[o * 128:(o + 1) * 128, :])
            wts[o] = w_t

        def x_load(c, queues=None):
            # per-k descriptors: chunk-0 chains consume tiles
            # k-progressively as they land.
            s0 = c * CH
            xts = []
            for k in range(KT):
                x_t = xpool.tile([128, CH], F16, tag=f"x{k}")
                q = queues[k % len(queues)] if queues else nc.sync
                q.dma_start(x_t[:], xt[k, :, s0:s0 + CH])
                xts.append(x_t)
            xts_by_chunk[s0] = [
                (lambda t: (lambda lo, hi: t[:, lo:hi]))(x_t) for x_t in xts]

        def x_load_big(s0, w):
            # steady-state chunks: ONE descriptor for the whole chunk
            # (8 strided k-blocks) => one DMA semaphore instead of 8 on
            # the Tensor queue.
            xbig = xpool.tile([128, KT * CH], F16, tag="xbig")
            nc.sync.dma_start(
                xbig[:, :KT * w].rearrange("p (k t) -> p k t", k=KT),
                xt[:, :, s0:s0 + w].rearrange("k p t -> p k t"))
            xts_by_chunk[s0] = [
                (lambda kk: (lambda lo, hi:
                             xbig[:, kk * w + lo:kk * w + hi]))(k)
                for k in range(KT)]

        wts = [None] * (2 * JT)
        xts_by_chunk = {}
        # Chunk-0's end is data-bound (w pair + ~1.3 MB) no matter how
        # early the first matmul fires, and the DMA queues ramp slowly in
        # the first few us, so keep the simple smooth split: w tiles whole
        # on Sync, x chunk 0 alternating between Sync and Scalar, gate
        # w(j=0) on Scalar.  (Tried and regressed: per-k W slices [256 B
        # runs collapse DMA efficiency], x0.k0-first-on-Scalar with w(0)
        # on GpSimd [Sync ramp starves the even x tiles].)
        w_load(JT + 0)
        w_load(0, nc.scalar)
        # (A third queue for chunk-0 x was tried and regresses: the 8 DMA
        # engines are shared across queues, so it steals from Sync/Scalar.)
        x_load(0, queues=[nc.sync, nc.scalar])
        for j in range(1, 6):
            w_load(JT + j), w_load(j)
        x_load(1)
        for j in range(6, 8):
            w_load(JT + j), w_load(j)

        # Const copies: first ACT op needs bias_t at ~17 us.
        bias_t = cpool.tile([128, 2 * JT], F32, tag="bias")
        nc.scalar.copy(bias_t[:], bias_d[:])
        nbias_t = cpool.tile([128, 2 * JT], F32, tag="nbias")
        nc.scalar.copy(nbias_t[:], nbias_d[:])
        h0_t = cpool.tile([128, JT], F32, tag="h0")
        nc.vector.tensor_copy(h0_t[:], h0_d[:])

        prev_h = [None] * JT
        g7pool = ctx.enter_context(tc.tile_pool(name="g7", bufs=1))

        def hidden_part(s0, w, j, sub, pool=None):
            """Hidden-half matmuls + post-ops (sg/relu/g) for the chain
            over time columns [s0, s0+w).  Returns a g-lookup closure.
            `pool` pins the g tiles in a dedicated pool so they survive
            the spool rotation (used to hoist the last chain's hidden
            work out of the end-of-kernel ACT queue)."""
            xl = xts_by_chunk[s0]
            ph = ppool.tile([128, CH], F32, tag="psum", name="ph")
            for k in range(KT):
                nc.tensor.matmul(
                    ph[:, :w],
                    lhsT=wts[JT + j][:, k * 128:(k + 1) * 128],
                    rhs=xl[k](0, w),
                    start=(k == 0),
                    stop=(k == KT - 1),
                )
            tiles = []
            for f0 in range(0, w, sub):
                fs = slice(f0, f0 + sub)
                sg_t = spool.tile([128, sub], F32, tag="sg")
                nc.scalar.activation(sg_t[:], ph[:, fs], AF.Sigmoid,
                                     bias=bias_t[:, JT + j:JT + j + 1],
                                     scale=1.0)
                r_t = spool.tile([128, sub], F32, tag="r")
                nc.scalar.activation(r_t[:], ph[:, fs], AF.Relu,
                                     bias=bias_t[:, JT + j:JT + j + 1],
                                     scale=1.0)
                # g = min(sigmoid(hidden), 0.5) + relu(hidden)
                if pool is None:
                    g_t = spool.tile([128, sub], F32, tag="g")
                else:
                    g_t = pool.tile([128, sub], F32, tag=f"g7_{f0}",
                                    name="g7_t")
                nc.vector.scalar_tensor_tensor(g_t[:], sg_t[:], 0.5,
                                               r_t[:], op0=OP.min,
                                               op1=OP.add)
                tiles.append((f0, sub, g_t))

            def g_at(f0, w):
                for t0, tw, t in tiles:
                    if t0 <= f0 and f0 + w <= t0 + tw:
                        return t[:, f0 - t0:f0 - t0 + w]
                raise KeyError((f0, w))
            return g_at

        def gate_part(s0, w, j, sub, g_at, gate_split=1, a_on_dve=False,
                      scan_q=None):
            """Gate-half matmuls + a/z/b/scan/store for the chain over
            time columns [s0, s0+w).  `gate_split` > 1 runs the gate
            matmuls as that many column sub-chains so the earlier blocks'
            post-ops overlap the later blocks' matmuls, and `a_on_dve`
            computes a = 1-z on the DVE instead of a second ACT sigmoid
            (both shorten the serial post-matmul tail; used for the very
            last chain only)."""
            xl = xts_by_chunk[s0]
            pg = ppool.tile([128, CH], F32, tag="psum", name="pg")
            gw = w // gate_split
            for lo in range(0, w, gw):
                for k in range(KT):
                    nc.tensor.matmul(
                        pg[:, lo:lo + gw],
                        lhsT=wts[j][:, k * 128:(k + 1) * 128],
                        rhs=xl[k](lo, lo + gw),
                        start=(k == 0),
                        stop=(k == KT - 1),
                    )
            h_t = hpool.tile([128, CH], F32, tag=f"h{j}")
            for f0 in range(0, w, sub):
                fs = slice(f0, f0 + sub)
                z_t = spool.tile([128, sub], F32, tag="z")
                nc.scalar.activation(z_t[:], pg[:, fs], AF.Sigmoid,
                                     bias=bias_t[:, j:j + 1], scale=1.0)
                a_t = abpool.tile([128, sub], F32, tag="a")
                if a_on_dve:
                    nc.vector.tensor_scalar(a_t[:], z_t[:], -1.0, 1.0,
                                            op0=OP.mult, op1=OP.add)
                else:
                    nc.scalar.activation(a_t[:], pg[:, fs], AF.Sigmoid,
                                         bias=nbias_t[:, j:j + 1], scale=-1.0)
                b_t = abpool.tile([128, sub], F32, tag="b")
                nc.vector.tensor_mul(b_t[:], z_t[:], g_at(f0, sub))
                # ---- scan: h = a*h_prev + b along time
                if f0 == 0:
                    if s0 == 0:
                        init = h0_t[:, j:j + 1]
                    else:
                        pt, pw = prev_h[j]
                        init = pt[:, pw - 1:pw]
                else:
                    init = h_t[:, f0 - 1:f0]
                (scan_q or nc.vector).tensor_tensor_scan(
                    h_t[:, fs], a_t[:], b_t[:], init,
                    op0=OP.mult, op1=OP.add)
            prev_h[j] = (h_t, w)
            # GpSimd's end-of-kernel DRAIN detects DMA completion
            # slowly (~6 us); keep the final chunks' stores on Sync
            # (idle by then) so the kernel end isn't gated on it.
            # The final chunk stores per-sub so the last transfer
            # is short.
            last = s0 + w == S
            out_q = nc.gpsimd if s0 + w <= S - 2 * CH else nc.sync
            if last:
                for f0 in range(0, w, sub):
                    out_q.dma_start(
                        out[j * 128:(j + 1) * 128, s0 + f0:s0 + f0 + sub],
                        h_t[:, f0:f0 + sub])
            else:
                out_q.dma_start(out[j * 128:(j + 1) * 128, s0:s0 + w],
                                h_t[:, :w])

        def gate_part_tail(s0, w, j, g_at):
            """Final chain's gate half: each 256-column block gets its OWN
            psum tile (with a shared tile the first block's z waits for
            ALL 16 matmuls -- psum dependencies are whole-tile granular),
            so block 1's z/a/b/scan overlap block 2's matmuls.  a = 1-z
            runs on the DVE; everything after the last matmul is one
            z -> a/b -> scan -> store chain."""
            xl = xts_by_chunk[s0]
            h_t = hpool.tile([128, CH], F32, tag=f"h{j}")
            # Asymmetric blocks: the final block is only 128 columns so
            # the serial chain after the very last matmul is as short as
            # possible; the 384-block's post-ops run under the 128-block's
            # matmuls.
            blocks = ((0, 3 * w // 4), (3 * w // 4, w // 4))
            for hi, (lo, bw) in enumerate(blocks):
                pg = ppool.tile([128, CH], F32, tag="psum", name=f"pgt{hi}")
                for k in range(KT):
                    nc.tensor.matmul(
                        pg[:, :bw],
                        lhsT=wts[j][:, k * 128:(k + 1) * 128],
                        rhs=xl[k](lo, lo + bw),
                        start=(k == 0),
                        stop=(k == KT - 1),
                    )
                z_t = spool.tile([128, bw], F32, tag="z")
                nc.scalar.activation(z_t[:], pg[:, :bw], AF.Sigmoid,
                                     bias=bias_t[:, j:j + 1], scale=1.0)
                a_t = abpool.tile([128, bw], F32, tag="a")
                nc.vector.tensor_scalar(a_t[:], z_t[:], -1.0, 1.0,
                                        op0=OP.mult, op1=OP.add)
                b_t = abpool.tile([128, bw], F32, tag="b")
                nc.vector.tensor_mul(b_t[:], z_t[:], g_at(lo, bw))
                if lo == 0:
                    pt, pw = prev_h[j]
                    init = pt[:, pw - 1:pw]
                else:
                    init = h_t[:, lo - 1:lo]
                nc.vector.tensor_tensor_scan(h_t[:, lo:lo + bw], a_t[:],
                                             b_t[:], init, op0=OP.mult,
                                             op1=OP.add)
                nc.sync.dma_start(
                    out[j * 128:(j + 1) * 128, s0 + lo:s0 + lo + bw],
                    h_t[:, lo:lo + bw])
            prev_h[j] = (h_t, w)

        def chain(s0, w, j, sub, gate_split=1, a_on_dve=False,
                  scan_q=None):
            gate_part(s0, w, j, sub, hidden_part(s0, w, j, sub),
                      gate_split, a_on_dve, scan_q)

        # Chunks 0/1 (512 wide): single-chunk chains, interleaved so
        # chains line up with the W/x arrival schedule.
        for c, j in ([(0, j) for j in range(6)] + [(1, j) for j in range(4)]
                     + [(0, 6), (0, 7)] + [(1, j) for j in range(4, 8)]):
            chain(c * CH, CH, j, CH)
        # Steady-state 512-wide chunks (the ISA rejects matmul outputs
        # > 512 elements -- s3d3_mm_num_elements -- so wider chunks are
        # not possible).  The last chain splits its gate matmuls and
        # post-ops so the serial a/z/mul/scan/store tail after the last
        # matmul is short.
        for s0, w in [(c * CH, CH) for c in range(2, NCHUNK)]:
            x_load_big(s0, w)
            if s0 + w == S:
                # Final chunk: hoist the last chain's hidden half to the
                # front (its sg/relu would otherwise sit in the ACT FIFO
                # ahead of the tail z ops), then run its gate half last
                # with per-block psum tiles.
                g7 = hidden_part(s0, w, JT - 1, CH, pool=g7pool)
                for j in range(JT - 1):
                    chain(s0, w, j, CH)
                gate_part_tail(s0, w, JT - 1, g7)
            else:
                for j in range(JT):
                    chain(s0, w, j, CH)

    nc.compile()
    return nc


def _run(inputs, trace=False, **spmd_kwargs):
    x = np.asarray(inputs["x"], dtype=np.float32)
    h = np.asarray(inputs["h"], dtype=np.float32)
    W = np.asarray(inputs["W"], dtype=np.float32)
    b = np.asarray(inputs["b"], dtype=np.float32)

    xt_all = np.ascontiguousarray(x.transpose(0, 2, 1)).astype(np.float16).reshape(
        B, KT, 128, S)                                             # (B,KT,128,S)
    # wp[o*128+p, k*128+c] = W[o*128+c, k*128+p]
    WP = np.ascontiguousarray(
        W.reshape(2 * JT, 128, KT, 128).transpose(0, 3, 2, 1)
        .reshape(2 * DH, DIN)).astype(np.float16)
    bias_t = np.ascontiguousarray(b.reshape(2 * JT, 128).T)        # (128, 2JT)
    nbias_t = np.ascontiguousarray(-bias_t)
    h0_all = np.ascontiguousarray(
        h[:, 0, :].reshape(B, JT, 128).transpose(0, 2, 1))         # (B, 128, JT)

    if "prog" not in _prog_cache:
        _prog_cache["prog"] = _build_program()
    nc = _prog_cache["prog"]

    in_maps = [
        {"xt": xt_all[c], "wp": WP, "bias": bias_t, "nbias": nbias_t,
         "h0": h0_all[c]}
        for c in range(B)
    ]
    res = run_bass_kernel_spmd(nc, in_maps, list(range(B)), trace=trace,
                               **spmd_kwargs)
    out = np.stack([res.results[c]["out"].T for c in range(B)], axis=0)
    return np.ascontiguousarray(out), res


def kernel(**inputs) -> np.ndarray:
    return _run(inputs)[0]

